# revision 7
# baseline (speedup 1.0000x reference)
"""AdaptiveStdPool2d kernel for Trainium2 (8 NeuronCores, data-parallel).

Input  x: [32, 64, 512, 80] f32
Output:   [32, 64, 8, 10] f32  (mean/std interleaved along height)

Math: per (b, c), split H=512 into 4 windows of 128 and W=80 into 10
windows of 8; out[b,c,2*oh,ow] = mean of 128x8 window, out[b,c,2*oh+1,ow]
= sqrt(biased_var + 1e-14).

Strategy (per core, shard = 4 batches -> 256 (b,c) rows):
- partition dim = (b,c) row (2 tiles of 128), free dim = flattened (h,w).
- per (bc-tile, oh): one big contiguous DMA slab [128, 10240] (40 KB/row).
- sums: ONE DVE tensor_reduce (axis=XY) over [p, ow, w, r] -> [p, 10].
- sumsq: per window, ACT Square with accum_out -> [p, 1] (10 per slab);
  ACT and DVE each stream the slab once, both under the DMA slab time.
- tail: mean = sums/1024, var = sumsq/1024 - mean^2, std = sqrt(var+eps),
  pack [oh, kind, ow] rows, 1 DMA out per bc-tile.
"""

import os
import numpy as np

B, C, H, W = 32, 64, 512, 80
OUT_H2, OUT_W = 4, 10
WH, WW = H // OUT_H2, W // OUT_W  # 128, 8
EPS = 1e-14
NWIN = WH * WW                   # 1024 elements per window

N_CORES = 8
B_SH = B // N_CORES          # 4 batches per core
BC = B_SH * C                # 256 rows per core
HW = H * W                   # 40960
SLAB = WH * W                # 10240 elements per (oh) slab
OUT_FREE = 2 * OUT_H2 * OUT_W  # 80 output elements per (b,c)

_CACHE = {}
LAST_RESULTS = None


def _build():
    import concourse.bacc as bacc
    import concourse.tile as tile
    from concourse import mybir

    nc = bacc.Bacc("TRN2", target_bir_lowering=False, debug=False)

    x = nc.dram_tensor("x", [BC, HW], mybir.dt.float32, kind="ExternalInput")
    out = nc.dram_tensor("out", [BC, OUT_FREE], mybir.dt.float32,
                         kind="ExternalOutput")

    P = 128
    NT = BC // P  # 2 bc-tiles
    INV_N = 1.0 / NWIN

    with tile.TileContext(nc) as tc:
        with (
            tc.tile_pool(name="slabs", bufs=4) as slabs,
            tc.tile_pool(name="scratch", bufs=3) as scratch_pool,
            tc.tile_pool(name="small", bufs=4) as small,
            tc.tile_pool(name="res", bufs=2) as res_pool,
            tc.tile_pool(name="singles", bufs=1) as singles,
        ):
            eps_t = singles.tile([P, 1], mybir.dt.float32)
            nc.vector.memset(eps_t, EPS)

            for t in range(NT):
                res = res_pool.tile([P, OUT_H2, 2, OUT_W], mybir.dt.float32)
                for oh in range(OUT_H2):
                    # Split the very last slab into two r-halves so its
                    # compute overlaps its own DMA (shortens the drain tail).
                    last = (t == NT - 1) and (oh == OUT_H2 - 1)
                    nh = 2 if last else 1
                    hlen = SLAB // nh
                    slab = slabs.tile([P, SLAB], mybir.dt.float32)
                    for h in range(nh):
                        nc.sync.dma_start(
                            out=slab[:, h * hlen:(h + 1) * hlen],
                            in_=x[t * P:(t + 1) * P,
                                  oh * SLAB + h * hlen:
                                  oh * SLAB + (h + 1) * hlen],
                        )
                    sums_h = small.tile([P, nh * OUT_W], mybir.dt.float32)
                    sqs_h = small.tile([P, nh * OUT_W], mybir.dt.float32)
                    for h in range(nh):
                        # [p, (r w)] -> [p, ow, r, w] (steps: ow=8, r=80, w=1)
                        # innermost = contiguous w for streaming-friendly reads
                        half_v = slab[:, h * hlen:(h + 1) * hlen].rearrange(
                            "p (r ow w) -> p ow r w", ow=OUT_W, w=WW)
                        nc.vector.tensor_reduce(
                            out=sums_h[:, h * OUT_W:(h + 1) * OUT_W],
                            in_=half_v,
                            axis=mybir.AxisListType.XY,
                            op=mybir.AluOpType.add,
                        )
                        for ow in range(OUT_W):
                            sq_scr = scratch_pool.tile([P, WH, WW],
                                                       mybir.dt.float32)
                            nc.scalar.activation(
                                out=sq_scr[:, :WH // nh, :],
                                in_=half_v[:, ow],
                                func=mybir.ActivationFunctionType.Square,
                                accum_out=sqs_h[:, h * OUT_W + ow:
                                                h * OUT_W + ow + 1],
                            )
                    if nh == 1:
                        sums, sqs = sums_h, sqs_h
                    else:
                        sums = small.tile([P, OUT_W], mybir.dt.float32)
                        sqs = small.tile([P, OUT_W], mybir.dt.float32)
                        nc.vector.tensor_reduce(
                            out=sums[:],
                            in_=sums_h.rearrange("p (h ow) -> p ow h", h=nh),
                            axis=mybir.AxisListType.X,
                            op=mybir.AluOpType.add,
                        )
                        nc.vector.tensor_reduce(
                            out=sqs[:],
                            in_=sqs_h.rearrange("p (h ow) -> p ow h", h=nh),
                            axis=mybir.AxisListType.X,
                            op=mybir.AluOpType.add,
                        )
                    # mean = sums / N
                    nc.vector.tensor_scalar_mul(res[:, oh, 0, :], sums[:],
                                                INV_N)
                    # mean^2
                    m2 = small.tile([P, OUT_W], mybir.dt.float32)
                    nc.vector.tensor_mul(m2[:], res[:, oh, 0, :],
                                         res[:, oh, 0, :])
                    # var = sumsq/N - mean^2
                    var = small.tile([P, OUT_W], mybir.dt.float32)
                    nc.vector.scalar_tensor_tensor(
                        out=var[:],
                        in0=sqs[:],
                        scalar=INV_N,
                        in1=m2[:],
                        op0=mybir.AluOpType.mult,
                        op1=mybir.AluOpType.subtract,
                    )
                    # std = sqrt(var + eps)
                    nc.scalar.activation(
                        out=res[:, oh, 1, :],
                        in_=var[:],
                        func=mybir.ActivationFunctionType.Sqrt,
                        bias=eps_t[:],
                        scale=1.0,
                    )
                nc.sync.dma_start(out=out[t * P:(t + 1) * P, :], in_=res[:])
    nc.compile()
    return nc


def kernel(x: np.ndarray) -> np.ndarray:
    global LAST_RESULTS
    from concourse.bass_utils import run_bass_kernel_spmd

    if "nc" not in _CACHE:
        _CACHE["nc"] = _build()
    nc = _CACHE["nc"]

    x = np.ascontiguousarray(np.asarray(x, dtype=np.float32))
    in_maps = [
        {"x": x[i * B_SH:(i + 1) * B_SH].reshape(BC, HW)}
        for i in range(N_CORES)
    ]
    trace = bool(int(os.environ.get("KERNEL_TRACE", "0")))
    res = run_bass_kernel_spmd(nc, in_maps, core_ids=list(range(N_CORES)),
                               trace=trace)
    LAST_RESULTS = res
    out = np.concatenate(
        [res.results[i]["out"].reshape(B_SH, C, 2 * OUT_H2, OUT_W)
         for i in range(N_CORES)],
        axis=0,
    )
    return out


# revision 8
# speedup vs baseline: 1.0298x; 1.0298x over previous
"""AdaptiveStdPool2d kernel for Trainium2 (8 NeuronCores, data-parallel).

Input  x: [32, 64, 512, 80] f32
Output:   [32, 64, 8, 10] f32  (mean/std interleaved along height)

Math: per (b, c), split H=512 into 4 windows of 128 and W=80 into 10
windows of 8; out[b,c,2*oh,ow] = mean of 128x8 window, out[b,c,2*oh+1,ow]
= sqrt(biased_var + 1e-14).

Strategy (per core, shard = 4 batches -> 256 (b,c) rows):
- partition dim = (b,c) row (2 tiles of 128), free dim = flattened (h,w).
- per (bc-tile, oh): one big contiguous DMA slab [128, 10240] (40 KB/row).
- sums: ONE DVE tensor_reduce (axis=XY) over [p, ow, w, r] -> [p, 10].
- sumsq: per window, ACT Square with accum_out -> [p, 1] (10 per slab);
  ACT and DVE each stream the slab once, both under the DMA slab time.
- tail: mean = sums/1024, var = sumsq/1024 - mean^2, std = sqrt(var+eps),
  pack [oh, kind, ow] rows, 1 DMA out per bc-tile.
"""

import os
import numpy as np

B, C, H, W = 32, 64, 512, 80
OUT_H2, OUT_W = 4, 10
WH, WW = H // OUT_H2, W // OUT_W  # 128, 8
EPS = 1e-14
NWIN = WH * WW                   # 1024 elements per window

N_CORES = 8
B_SH = B // N_CORES          # 4 batches per core
BC = B_SH * C                # 256 rows per core
HW = H * W                   # 40960
SLAB = WH * W                # 10240 elements per (oh) slab
OUT_FREE = 2 * OUT_H2 * OUT_W  # 80 output elements per (b,c)

_CACHE = {}
LAST_RESULTS = None


def _build():
    import concourse.bacc as bacc
    import concourse.tile as tile
    from concourse import mybir

    nc = bacc.Bacc("TRN2", target_bir_lowering=False, debug=False)

    x = nc.dram_tensor("x", [BC, HW], mybir.dt.float32, kind="ExternalInput")
    out = nc.dram_tensor("out", [BC, OUT_FREE], mybir.dt.float32,
                         kind="ExternalOutput")

    P = 128
    NT = BC // P  # 2 bc-tiles
    INV_N = 1.0 / NWIN

    with tile.TileContext(nc) as tc:
        with (
            tc.tile_pool(name="slabs", bufs=4) as slabs,
            tc.tile_pool(name="scratch", bufs=3) as scratch_pool,
            tc.tile_pool(name="small", bufs=4) as small,
            tc.tile_pool(name="res", bufs=2) as res_pool,
            tc.tile_pool(name="singles", bufs=1) as singles,
        ):
            eps_t = singles.tile([P, 1], mybir.dt.float32)
            nc.vector.memset(eps_t, EPS)

            for t in range(NT):
                res = res_pool.tile([P, OUT_H2, 2, OUT_W], mybir.dt.float32)
                for oh in range(OUT_H2):
                    nh = 1
                    hlen = SLAB // nh
                    slab = slabs.tile([P, SLAB], mybir.dt.float32)
                    for h in range(nh):
                        nc.sync.dma_start(
                            out=slab[:, h * hlen:(h + 1) * hlen],
                            in_=x[t * P:(t + 1) * P,
                                  oh * SLAB + h * hlen:
                                  oh * SLAB + (h + 1) * hlen],
                        )
                    sums_h = small.tile([P, nh * OUT_W], mybir.dt.float32)
                    sqs_h = small.tile([P, nh * OUT_W], mybir.dt.float32)
                    for h in range(nh):
                        # [p, (r w)] -> [p, ow, r, w] (steps: ow=8, r=80, w=1)
                        # innermost = contiguous w for streaming-friendly reads
                        half_v = slab[:, h * hlen:(h + 1) * hlen].rearrange(
                            "p (r ow w) -> p ow r w", ow=OUT_W, w=WW)
                        nc.vector.tensor_reduce(
                            out=sums_h[:, h * OUT_W:(h + 1) * OUT_W],
                            in_=half_v,
                            axis=mybir.AxisListType.XY,
                            op=mybir.AluOpType.add,
                        )
                        for ow in range(OUT_W):
                            sq_scr = scratch_pool.tile([P, WH, WW],
                                                       mybir.dt.float32)
                            nc.scalar.activation(
                                out=sq_scr[:, :WH // nh, :],
                                in_=half_v[:, ow],
                                func=mybir.ActivationFunctionType.Square,
                                accum_out=sqs_h[:, h * OUT_W + ow:
                                                h * OUT_W + ow + 1],
                            )
                    if nh == 1:
                        sums, sqs = sums_h, sqs_h
                    else:
                        sums = small.tile([P, OUT_W], mybir.dt.float32)
                        sqs = small.tile([P, OUT_W], mybir.dt.float32)
                        nc.vector.tensor_reduce(
                            out=sums[:],
                            in_=sums_h.rearrange("p (h ow) -> p ow h", h=nh),
                            axis=mybir.AxisListType.X,
                            op=mybir.AluOpType.add,
                        )
                        nc.vector.tensor_reduce(
                            out=sqs[:],
                            in_=sqs_h.rearrange("p (h ow) -> p ow h", h=nh),
                            axis=mybir.AxisListType.X,
                            op=mybir.AluOpType.add,
                        )
                    # mean = sums / N
                    nc.vector.tensor_scalar_mul(res[:, oh, 0, :], sums[:],
                                                INV_N)
                    # mean^2
                    m2 = small.tile([P, OUT_W], mybir.dt.float32)
                    nc.vector.tensor_mul(m2[:], res[:, oh, 0, :],
                                         res[:, oh, 0, :])
                    # var = sumsq/N - mean^2
                    var = small.tile([P, OUT_W], mybir.dt.float32)
                    nc.vector.scalar_tensor_tensor(
                        out=var[:],
                        in0=sqs[:],
                        scalar=INV_N,
                        in1=m2[:],
                        op0=mybir.AluOpType.mult,
                        op1=mybir.AluOpType.subtract,
                    )
                    # std = sqrt(var + eps)
                    nc.scalar.activation(
                        out=res[:, oh, 1, :],
                        in_=var[:],
                        func=mybir.ActivationFunctionType.Sqrt,
                        bias=eps_t[:],
                        scale=1.0,
                    )
                nc.sync.dma_start(out=out[t * P:(t + 1) * P, :], in_=res[:])
    nc.compile()
    return nc


def kernel(x: np.ndarray) -> np.ndarray:
    global LAST_RESULTS
    from concourse.bass_utils import run_bass_kernel_spmd

    if "nc" not in _CACHE:
        _CACHE["nc"] = _build()
    nc = _CACHE["nc"]

    x = np.ascontiguousarray(np.asarray(x, dtype=np.float32))
    in_maps = [
        {"x": x[i * B_SH:(i + 1) * B_SH].reshape(BC, HW)}
        for i in range(N_CORES)
    ]
    trace = bool(int(os.environ.get("KERNEL_TRACE", "0")))
    res = run_bass_kernel_spmd(nc, in_maps, core_ids=list(range(N_CORES)),
                               trace=trace)
    LAST_RESULTS = res
    out = np.concatenate(
        [res.results[i]["out"].reshape(B_SH, C, 2 * OUT_H2, OUT_W)
         for i in range(N_CORES)],
        axis=0,
    )
    return out
